# revision 23
# baseline (speedup 1.0000x reference)
"""BlockDropout kernel for TRN2 (Bass/Tile), data-parallel over 8 NeuronCores.

Problem: z [128, 256, 1024] f32, noise [128, 1024] f32, fallback_idx [128] int.
  mask[b, d] = (noise[b, d] < 0.8); if a row of mask is all zero, force
  mask[b, fallback_idx[b]] = 1.  out[b, m, d] = mask[b, d] * z[b, m, d].

Sharding: batch dim split 8 ways (16 batches per core); no communication.

The force-nonzero fallback is folded into the noise tensor on the host (if a
row of noise is entirely >= 0.8, noise[b, fallback_idx[b]] is set to -1.0,
which forces mask[b, fallback_idx[b]] = 1 on device) — identical to the
reference semantics, and it keeps the device kernel a pure
compare + broadcast + multiply.

Per-core device kernel:
  - mask = (noise < 0.8) computed on DVE straight to bf16 (0/1 exact),
  - mask rows flattened to partition 0 with one SBUF->SBUF DMA,
  - per batch, the mask row is broadcast across the 128 SBUF partitions with
    K=1 bf16 matmuls on the (otherwise idle) PE into PSUM,
  - per batch, one [128, 2048] f32 tile holds all of z[b] (each partition has
    two of the 256 M-rows), loaded with a single 1 MiB DMA, multiplied on DVE
    against the PSUM mask, stored with a single 1 MiB DMA.
Loads are issued from SP (nc.sync) and stores from ACT (nc.scalar) so the two
HWDGE rings don't head-of-line block each other.
"""

import numpy as np

B, M, D = 128, 256, 1024
NCORES = 8
B_LOC = B // NCORES  # 16 batches per core
FREE = 2 * D         # 2048: two M-rows per SBUF partition => z[b] is [128, FREE]
KEEP = 0.8           # 1 - p_drop

_NC_CACHE = {}


def _build_bass_fat(reps=1):
    """2-batches-per-tile variant: [128, 4096] tiles (2 MiB DMAs), 8 loop
    iterations. Partition p holds M-rows 4p..4p+3 of a 512-row (2-batch)
    block, so partitions 0..63 belong to batch 2t and 64..127 to batch 2t+1;
    the PSUM mask tile is filled per 64-partition half (base partitions 0/64
    are both legal matmul output bases)."""
    import contextlib

    import concourse.bass as bass
    import concourse.mybir as mybir
    import concourse.tile as tile
    from concourse import bacc

    f32 = mybir.dt.float32
    bf16 = mybir.dt.bfloat16
    FAT = 2 * FREE  # 4096
    nt = B_LOC // 2  # 8 tiles; tile t covers batches t (partitions 0..63)
    #                  and t+8 (partitions 64..127)
    nc = bacc.Bacc(
        "TRN2", target_bir_lowering=False, debug=False, num_devices=NCORES
    )
    # [half, tile, partition-in-half, free]: natural reshape of [16, 256, 1024]
    z_d = nc.dram_tensor("z", [2, nt, 64, FAT], f32, kind="ExternalInput")
    noise_d = nc.dram_tensor("noise", [B_LOC, D], f32, kind="ExternalInput")
    out_d = nc.dram_tensor("out", [2, nt, 64, FAT], f32, kind="ExternalOutput")

    with tile.TileContext(nc) as tc:
        with (
            tc.tile_pool(name="const", bufs=1) as cpool,
            tc.tile_pool(name="zp", bufs=4) as zpool,
            tc.tile_pool(name="op", bufs=4) as opool,
            tc.tile_pool(name="mp", bufs=2, space=bass.MemorySpace.PSUM) as mpool,
        ):
            pre_z = {}
            if reps == 1:
                for t in range(2):
                    zt = zpool.tile([128, FAT], f32, tag="zt")
                    nc.sync.dma_start(zt[:], z_d.ap()[:, t, :, :])
                    pre_z[t] = zt

            noise_t = cpool.tile([B_LOC, D], f32)
            nc.sync.dma_start(noise_t[:], noise_d.ap())
            # selector for the K=2 broadcast matmul: out[p] gets batch t for
            # p<64 and batch t+8 for p>=64. Compute ops can only start at
            # partition 0/32/64/96, so build the two rows in a staging row and
            # DMA them into place (DMA has no start-partition restriction).
            sel_t = cpool.tile([2, 128], bf16)
            selstage_t = cpool.tile([1, 192], bf16)
            nc.vector.memset(selstage_t[0:1, 0:64], 0.0)
            nc.vector.memset(selstage_t[0:1, 64:128], 1.0)
            nc.vector.memset(selstage_t[0:1, 128:192], 0.0)
            nc.sync.dma_start(sel_t[0:1, :], selstage_t[0:1, 64:192])
            nc.sync.dma_start(sel_t[1:2, :], selstage_t[0:1, 0:128])

            maskf_t = cpool.tile([B_LOC, D], bf16)
            nc.vector.tensor_scalar(
                maskf_t[:], noise_t[:], KEEP, None, mybir.AluOpType.is_lt
            )
            # maskpair[0, t*D+d] = mask[t, d]; maskpair[1, t*D+d] = mask[t+8, d]
            maskpair_t = cpool.tile([2, nt * D], bf16)
            nc.sync.dma_start(maskpair_t[0:1, :], maskf_t[0:nt, :])
            nc.sync.dma_start(maskpair_t[1:2, :], maskf_t[nt:B_LOC, :])

            loop_cm = (
                tc.For_i(0, reps, 1) if reps > 1 else contextlib.nullcontext()
            )
            with loop_cm:
                for t in range(nt):
                    zt = pre_z.pop(t, None)
                    if zt is None:
                        zt = zpool.tile([128, FAT], f32, tag="zt")
                        nc.sync.dma_start(zt[:], z_d.ap()[:, t, :, :])
                    # the mask is 1024-periodic along the free dim, so one
                    # [128, 2048] PSUM tile serves both 2048-wide halves
                    pm = mpool.tile([128, FREE], f32)
                    for j in range(4):
                        nc.tensor.matmul(
                            pm[:, j * 512 : (j + 1) * 512],
                            sel_t[0:2, :],
                            maskpair_t[
                                0:2,
                                t * D + (j % 2) * 512 : t * D + (j % 2) * 512 + 512,
                            ],
                            start=True,
                            stop=True,
                        )
                    ot = opool.tile([128, FAT], f32)
                    nc.vector.tensor_mul(ot[:, 0:FREE], zt[:, 0:FREE], pm[:])
                    nc.vector.tensor_mul(ot[:, FREE:FAT], zt[:, FREE:FAT], pm[:])
                    nc.scalar.dma_start(out_d.ap()[:, t, :, :], ot[:])
    nc.compile()
    return nc


def _build_bass(reps=1):
    """Build the per-core module. reps>1 wraps the batch loop in a dynamic
    For_i that redoes the same work (used only for benchmarking)."""
    import contextlib

    import concourse.bass as bass
    import concourse.mybir as mybir
    import concourse.tile as tile
    from concourse import bacc

    f32 = mybir.dt.float32
    bf16 = mybir.dt.bfloat16
    nc = bacc.Bacc(
        "TRN2", target_bir_lowering=False, debug=False, num_devices=NCORES
    )
    z_d = nc.dram_tensor("z", [B_LOC, 128, FREE], f32, kind="ExternalInput")
    noise_d = nc.dram_tensor("noise", [B_LOC, D], f32, kind="ExternalInput")
    out_d = nc.dram_tensor("out", [B_LOC, 128, FREE], f32, kind="ExternalOutput")

    with tile.TileContext(nc) as tc:
        with (
            tc.tile_pool(name="const", bufs=1) as cpool,
            tc.tile_pool(name="zp", bufs=6) as zpool,
            tc.tile_pool(name="op", bufs=6) as opool,
            tc.tile_pool(name="mp", bufs=2, space=bass.MemorySpace.PSUM) as mpool,
        ):
            # issue the first z loads before anything else so the DMA engines
            # saturate during the sequencer preamble + mask prep (single-shot
            # module only; the bench loop keeps all loads inside the body)
            pre_z = {}
            if reps == 1:
                for b in range(2):
                    zt = zpool.tile([128, FREE], f32, tag="zt")
                    nc.sync.dma_start(zt[:], z_d.ap()[b])
                    pre_z[b] = zt

            noise_t = cpool.tile([B_LOC, D], f32)
            nc.sync.dma_start(noise_t[:], noise_d.ap())
            ones_t = cpool.tile([1, 128], bf16)
            nc.vector.memset(ones_t[:], 1.0)

            # mask = (noise < 0.8) as 1.0/0.0, straight to bf16 (exact for 0/1;
            # bf16 runs 4x faster on the PE broadcast matmuls below)
            maskf_t = cpool.tile([B_LOC, D], bf16)
            nc.vector.tensor_scalar(
                maskf_t[:], noise_t[:], KEEP, None, mybir.AluOpType.is_lt
            )
            # flatten all mask rows onto partition 0 so matmul rhs reads are
            # at base partition 0 (HW requires base partition 0/32/64)
            maskrow_t = cpool.tile([1, B_LOC * D], bf16)
            nc.sync.dma_start(maskrow_t[0:1, :], maskf_t[:])

            loop_cm = (
                tc.For_i(0, reps, 1) if reps > 1 else contextlib.nullcontext()
            )
            with loop_cm:
                for b in range(B_LOC):
                    zt = pre_z.pop(b, None)
                    if zt is None:
                        zt = zpool.tile([128, FREE], f32, tag="zt")
                        nc.sync.dma_start(zt[:], z_d.ap()[b])
                    # broadcast mask row b across 128 partitions:
                    # ones[1,128].T @ mask[1,512]
                    pm = mpool.tile([128, FREE], f32)
                    for j in range(4):
                        nc.tensor.matmul(
                            pm[:, j * 512 : (j + 1) * 512],
                            ones_t[0:1, :],
                            maskrow_t[
                                0:1,
                                b * D + (j % 2) * 512 : b * D + (j % 2) * 512 + 512,
                            ],
                            start=True,
                            stop=True,
                        )
                    ot = opool.tile([128, FREE], f32)
                    if b == B_LOC - 1 and reps == 1:
                        # split the final multiply+store in halves so the tail
                        # store is half-size (shorter drain before the barrier)
                        nc.vector.tensor_mul(
                            ot[:, 0:D], zt[:, 0:D], pm[:, 0:D]
                        )
                        nc.scalar.dma_start(out_d.ap()[b][:, 0:D], ot[:, 0:D])
                        nc.vector.tensor_mul(
                            ot[:, D:FREE], zt[:, D:FREE], pm[:, D:FREE]
                        )
                        nc.scalar.dma_start(
                            out_d.ap()[b][:, D:FREE], ot[:, D:FREE]
                        )
                    else:
                        nc.vector.tensor_mul(ot[:], zt[:], pm[:])
                        nc.scalar.dma_start(out_d.ap()[b], ot[:])
    nc.compile()
    return nc


def get_nc():
    if "nc" not in _NC_CACHE:
        _NC_CACHE["nc"] = _build_bass()
    return _NC_CACHE["nc"]


def _precondition_noise(noise, fidx):
    """Fold the force-nonzero fallback into noise: rows whose mask would be
    all zero get noise[b, fidx[b]] = -1.0 (=> mask 1 at that position)."""
    noise = np.ascontiguousarray(np.asarray(noise, dtype=np.float32)).copy()
    keep = noise < np.float32(KEEP)
    dead = ~keep.any(axis=1)
    if dead.any():
        rows = np.nonzero(dead)[0]
        noise[rows, fidx[rows]] = -1.0
    return noise


def kernel(z, noise, fallback_idx):
    from concourse.bass_utils import run_bass_kernel_spmd

    z = np.ascontiguousarray(np.asarray(z, dtype=np.float32))
    fidx = np.asarray(fallback_idx).astype(np.int64)
    assert z.shape == (B, M, D) and fidx.shape == (B,)
    noise = _precondition_noise(noise, fidx)
    assert noise.shape == (B, D)

    nc = get_nc()
    in_maps = []
    for c in range(NCORES):
        sl = slice(c * B_LOC, (c + 1) * B_LOC)
        in_maps.append(
            {
                "z": z[sl].reshape(B_LOC, 128, FREE),
                "noise": noise[sl],
            }
        )
    res = run_bass_kernel_spmd(nc, in_maps, core_ids=list(range(NCORES)))
    outs = [r["out"].reshape(B_LOC, M, D) for r in res.results]
    return np.concatenate(outs, axis=0)


# revision 25
# speedup vs baseline: 1.1641x; 1.1641x over previous
"""BlockDropout kernel for TRN2 (Bass/Tile), data-parallel over 8 NeuronCores.

Problem: z [128, 256, 1024] f32, noise [128, 1024] f32, fallback_idx [128] int.
  mask[b, d] = (noise[b, d] < 0.8); if a row of mask is all zero, force
  mask[b, fallback_idx[b]] = 1.  out[b, m, d] = mask[b, d] * z[b, m, d].

Sharding: batch dim split 8 ways (16 batches per core); no communication.

The force-nonzero fallback is folded into the noise tensor on the host (if a
row of noise is entirely >= 0.8, noise[b, fallback_idx[b]] is set to -1.0,
which forces mask[b, fallback_idx[b]] = 1 on device) — identical to the
reference semantics, and it keeps the device kernel a pure
compare + broadcast + multiply.

Per-core device kernel:
  - mask = (noise < 0.8) computed on DVE straight to bf16 (0/1 exact),
  - mask rows flattened to partition 0 with one SBUF->SBUF DMA,
  - per batch, the mask row is broadcast across the 128 SBUF partitions with
    K=1 bf16 matmuls on the (otherwise idle) PE into PSUM,
  - per batch, one [128, 2048] f32 tile holds all of z[b] (each partition has
    two of the 256 M-rows), loaded with a single 1 MiB DMA, multiplied on DVE
    against the PSUM mask, stored with a single 1 MiB DMA.
Loads are issued from SP (nc.sync) and stores from ACT (nc.scalar) so the two
HWDGE rings don't head-of-line block each other.
"""

import numpy as np

B, M, D = 128, 256, 1024
NCORES = 8
B_LOC = B // NCORES  # 16 batches per core
FREE = 2 * D         # 2048: two M-rows per SBUF partition => z[b] is [128, FREE]
KEEP = 0.8           # 1 - p_drop

_NC_CACHE = {}


def _build_bass_fat(reps=1):
    """2-batches-per-tile variant: [128, 4096] tiles (2 MiB DMAs), 8 loop
    iterations. Partition p holds M-rows 4p..4p+3 of a 512-row (2-batch)
    block, so partitions 0..63 belong to batch 2t and 64..127 to batch 2t+1;
    the PSUM mask tile is filled per 64-partition half (base partitions 0/64
    are both legal matmul output bases)."""
    import contextlib

    import concourse.bass as bass
    import concourse.mybir as mybir
    import concourse.tile as tile
    from concourse import bacc

    f32 = mybir.dt.float32
    bf16 = mybir.dt.bfloat16
    FAT = 2 * FREE  # 4096
    nt = B_LOC // 2  # 8 tiles; tile t covers batches t (partitions 0..63)
    #                  and t+8 (partitions 64..127)
    nc = bacc.Bacc(
        "TRN2", target_bir_lowering=False, debug=False, num_devices=NCORES
    )
    # [half, tile, partition-in-half, free]: natural reshape of [16, 256, 1024]
    z_d = nc.dram_tensor("z", [2, nt, 64, FAT], f32, kind="ExternalInput")
    noise_d = nc.dram_tensor("noise", [B_LOC, D], f32, kind="ExternalInput")
    out_d = nc.dram_tensor("out", [2, nt, 64, FAT], f32, kind="ExternalOutput")

    with tile.TileContext(nc) as tc:
        with (
            tc.tile_pool(name="const", bufs=1) as cpool,
            tc.tile_pool(name="zp", bufs=4) as zpool,
            tc.tile_pool(name="op", bufs=4) as opool,
            tc.tile_pool(name="mp", bufs=2, space=bass.MemorySpace.PSUM) as mpool,
        ):
            pre_z = {}
            if reps == 1:
                for t in range(2):
                    zt = zpool.tile([128, FAT], f32, tag="zt")
                    nc.sync.dma_start(zt[:], z_d.ap()[:, t, :, :])
                    pre_z[t] = zt

            # all mask-prep DMAs go on the ACT ring: the SP ring is busy with
            # 2 MiB z loads from t=0 and small prep DMAs queued behind them
            # would head-of-line block the whole mask chain for ~20 us
            noise_t = cpool.tile([B_LOC, D], f32)
            nc.scalar.dma_start(noise_t[:], noise_d.ap())
            # selector for the K=2 broadcast matmul: out[p] gets batch t for
            # p<64 and batch t+8 for p>=64. Compute ops can only start at
            # partition 0/32/64/96, so build the two rows in a staging row and
            # DMA them into place (DMA has no start-partition restriction).
            sel_t = cpool.tile([2, 128], bf16)
            selstage_t = cpool.tile([1, 192], bf16)
            nc.vector.memset(selstage_t[0:1, 0:64], 0.0)
            nc.vector.memset(selstage_t[0:1, 64:128], 1.0)
            nc.vector.memset(selstage_t[0:1, 128:192], 0.0)
            nc.scalar.dma_start(sel_t[0:1, :], selstage_t[0:1, 64:192])
            nc.scalar.dma_start(sel_t[1:2, :], selstage_t[0:1, 0:128])

            maskf_t = cpool.tile([B_LOC, D], bf16)
            nc.vector.tensor_scalar(
                maskf_t[:], noise_t[:], KEEP, None, mybir.AluOpType.is_lt
            )
            # maskpair[0, t*D+d] = mask[t, d]; maskpair[1, t*D+d] = mask[t+8, d]
            maskpair_t = cpool.tile([2, nt * D], bf16)
            nc.scalar.dma_start(maskpair_t[0:1, :], maskf_t[0:nt, :])
            nc.scalar.dma_start(maskpair_t[1:2, :], maskf_t[nt:B_LOC, :])

            loop_cm = (
                tc.For_i(0, reps, 1) if reps > 1 else contextlib.nullcontext()
            )
            with loop_cm:
                for t in range(nt):
                    zt = pre_z.pop(t, None)
                    if zt is None:
                        zt = zpool.tile([128, FAT], f32, tag="zt")
                        nc.sync.dma_start(zt[:], z_d.ap()[:, t, :, :])
                    # the mask is 1024-periodic along the free dim, so one
                    # [128, 2048] PSUM tile serves both 2048-wide halves
                    pm = mpool.tile([128, FREE], f32)
                    for j in range(4):
                        nc.tensor.matmul(
                            pm[:, j * 512 : (j + 1) * 512],
                            sel_t[0:2, :],
                            maskpair_t[
                                0:2,
                                t * D + (j % 2) * 512 : t * D + (j % 2) * 512 + 512,
                            ],
                            start=True,
                            stop=True,
                        )
                    ot = opool.tile([128, FAT], f32)
                    nc.vector.tensor_mul(ot[:, 0:FREE], zt[:, 0:FREE], pm[:])
                    if t == nt - 1 and reps == 1:
                        # split the final store so its first half streams out
                        # while DVE finishes the second half
                        nc.scalar.dma_start(
                            out_d.ap()[:, t, :, 0:FREE], ot[:, 0:FREE]
                        )
                        nc.vector.tensor_mul(
                            ot[:, FREE:FAT], zt[:, FREE:FAT], pm[:]
                        )
                        nc.scalar.dma_start(
                            out_d.ap()[:, t, :, FREE:FAT], ot[:, FREE:FAT]
                        )
                    else:
                        nc.vector.tensor_mul(
                            ot[:, FREE:FAT], zt[:, FREE:FAT], pm[:]
                        )
                        nc.scalar.dma_start(out_d.ap()[:, t, :, :], ot[:])
    nc.compile()
    return nc


def _build_bass(reps=1):
    """Build the per-core module. reps>1 wraps the batch loop in a dynamic
    For_i that redoes the same work (used only for benchmarking)."""
    import contextlib

    import concourse.bass as bass
    import concourse.mybir as mybir
    import concourse.tile as tile
    from concourse import bacc

    f32 = mybir.dt.float32
    bf16 = mybir.dt.bfloat16
    nc = bacc.Bacc(
        "TRN2", target_bir_lowering=False, debug=False, num_devices=NCORES
    )
    z_d = nc.dram_tensor("z", [B_LOC, 128, FREE], f32, kind="ExternalInput")
    noise_d = nc.dram_tensor("noise", [B_LOC, D], f32, kind="ExternalInput")
    out_d = nc.dram_tensor("out", [B_LOC, 128, FREE], f32, kind="ExternalOutput")

    with tile.TileContext(nc) as tc:
        with (
            tc.tile_pool(name="const", bufs=1) as cpool,
            tc.tile_pool(name="zp", bufs=6) as zpool,
            tc.tile_pool(name="op", bufs=6) as opool,
            tc.tile_pool(name="mp", bufs=2, space=bass.MemorySpace.PSUM) as mpool,
        ):
            # issue the first z loads before anything else so the DMA engines
            # saturate during the sequencer preamble + mask prep (single-shot
            # module only; the bench loop keeps all loads inside the body)
            pre_z = {}
            if reps == 1:
                for b in range(2):
                    zt = zpool.tile([128, FREE], f32, tag="zt")
                    nc.sync.dma_start(zt[:], z_d.ap()[b])
                    pre_z[b] = zt

            noise_t = cpool.tile([B_LOC, D], f32)
            nc.sync.dma_start(noise_t[:], noise_d.ap())
            ones_t = cpool.tile([1, 128], bf16)
            nc.vector.memset(ones_t[:], 1.0)

            # mask = (noise < 0.8) as 1.0/0.0, straight to bf16 (exact for 0/1;
            # bf16 runs 4x faster on the PE broadcast matmuls below)
            maskf_t = cpool.tile([B_LOC, D], bf16)
            nc.vector.tensor_scalar(
                maskf_t[:], noise_t[:], KEEP, None, mybir.AluOpType.is_lt
            )
            # flatten all mask rows onto partition 0 so matmul rhs reads are
            # at base partition 0 (HW requires base partition 0/32/64)
            maskrow_t = cpool.tile([1, B_LOC * D], bf16)
            nc.sync.dma_start(maskrow_t[0:1, :], maskf_t[:])

            loop_cm = (
                tc.For_i(0, reps, 1) if reps > 1 else contextlib.nullcontext()
            )
            with loop_cm:
                for b in range(B_LOC):
                    zt = pre_z.pop(b, None)
                    if zt is None:
                        zt = zpool.tile([128, FREE], f32, tag="zt")
                        nc.sync.dma_start(zt[:], z_d.ap()[b])
                    # broadcast mask row b across 128 partitions:
                    # ones[1,128].T @ mask[1,512]
                    pm = mpool.tile([128, FREE], f32)
                    for j in range(4):
                        nc.tensor.matmul(
                            pm[:, j * 512 : (j + 1) * 512],
                            ones_t[0:1, :],
                            maskrow_t[
                                0:1,
                                b * D + (j % 2) * 512 : b * D + (j % 2) * 512 + 512,
                            ],
                            start=True,
                            stop=True,
                        )
                    ot = opool.tile([128, FREE], f32)
                    if b == B_LOC - 1 and reps == 1:
                        # split the final multiply+store in halves so the tail
                        # store is half-size (shorter drain before the barrier)
                        nc.vector.tensor_mul(
                            ot[:, 0:D], zt[:, 0:D], pm[:, 0:D]
                        )
                        nc.scalar.dma_start(out_d.ap()[b][:, 0:D], ot[:, 0:D])
                        nc.vector.tensor_mul(
                            ot[:, D:FREE], zt[:, D:FREE], pm[:, D:FREE]
                        )
                        nc.scalar.dma_start(
                            out_d.ap()[b][:, D:FREE], ot[:, D:FREE]
                        )
                    else:
                        nc.vector.tensor_mul(ot[:], zt[:], pm[:])
                        nc.scalar.dma_start(out_d.ap()[b], ot[:])
    nc.compile()
    return nc


def get_nc():
    if "nc" not in _NC_CACHE:
        _NC_CACHE["nc"] = _build_bass()
    return _NC_CACHE["nc"]


def _precondition_noise(noise, fidx):
    """Fold the force-nonzero fallback into noise: rows whose mask would be
    all zero get noise[b, fidx[b]] = -1.0 (=> mask 1 at that position)."""
    noise = np.ascontiguousarray(np.asarray(noise, dtype=np.float32)).copy()
    keep = noise < np.float32(KEEP)
    dead = ~keep.any(axis=1)
    if dead.any():
        rows = np.nonzero(dead)[0]
        noise[rows, fidx[rows]] = -1.0
    return noise


def kernel(z, noise, fallback_idx):
    from concourse.bass_utils import run_bass_kernel_spmd

    z = np.ascontiguousarray(np.asarray(z, dtype=np.float32))
    fidx = np.asarray(fallback_idx).astype(np.int64)
    assert z.shape == (B, M, D) and fidx.shape == (B,)
    noise = _precondition_noise(noise, fidx)
    assert noise.shape == (B, D)

    nc = get_nc()
    in_maps = []
    for c in range(NCORES):
        sl = slice(c * B_LOC, (c + 1) * B_LOC)
        in_maps.append(
            {
                "z": z[sl].reshape(B_LOC, 128, FREE),
                "noise": noise[sl],
            }
        )
    res = run_bass_kernel_spmd(nc, in_maps, core_ids=list(range(NCORES)))
    outs = [r["out"].reshape(B_LOC, M, D) for r in res.results]
    return np.concatenate(outs, axis=0)
